# revision 85
# baseline (speedup 1.0000x reference)
"""HDC binary attention kernel for 8 trn2 NeuronCores.

Problem: B,T,D = 4,2048,1024
    Q = sign(x * sign(bv_q)); K = sign(x * sign(bv_k)); V = x * sign(bv_v)
    scores = (Q @ K^T) / sqrt(D), causal
    out = sigmoid(4*scores) * causal_mask @ V

Math used by the kernel:
    sign(x*bq) = sign(x)*sign(bq), so with S = sign(x):
        scores[t,s] = sum_d (sq*S^T)[d,t] * (sk*S^T)[d,s] / 32.
    Host builds WQ = sq * S^T (moving operand) and WK = sk * S^T (stationary);
    +-1 entries are exact in fp8e4, and scoresT (s on partitions) accumulates
    in PSUM fp32 exactly. Both matmul phases run fp8 DoubleRow.

    The AV phase uses sigmoid(z) = (1 + tanh(z/2))/2:
        out[t] = P'[t] + sum_{s in chain(t)} th[t,s] * (V[s]/2)
    where chain(t) covers full 512-chunks 0..j for t in chunk j,
    P'[t] = 0.5 * sum_{s < 512(j+1)} V[s] (constant per chunk, host-built,
    fp16), th = tanh(scores/16) for s <= t and exactly -1 (additive -3e4
    PSUM bias before tanh) for masked in-chain positions so their V/2
    contribution cancels P'. th and V/2 are fp8 (DoubleRow), errors stay
    ~1e-2 relative. attnT/V are stored in s-pair layout [128, 2, cols] so a
    DoubleRow matmul contracts 256 s rows.

Sharding: 2 cores per batch. Each 512-row chunk of T is split in half:
    core parity 0 takes rows [512j, 512j+256), parity 1 takes [512j+256, 512j+512).
For SPMD uniformity the host permutes K/V rows for parity-1 cores (swapping the
halves of every 512-chunk) so that each core's q rows always sit at canonical
positions [512j, 512j+256); causal boundary handling is via host-built additive
masks. Each q group j attends to canonical s < 512*(j+1); full 512-chunks below
the boundary are permutation-invariant, the boundary chunk is masked explicitly.
"""

import numpy as np

B, T, D = 4, 2048, 1024
NQ = 1024          # q rows per core
NCORES = 8
ST = 16            # s-tiles of 128 rows
DT = 8             # d-tiles of 128
NG = 4             # q groups of 256 rows per core
NK = 8             # s-pair tiles of 256 rows

_CACHE = {}

# Emission schedule: "L:<name>" load, "SP:<ss>" scores pair (ss, ss+1),
# "A:<ts>" av tile ("AF" = fast HWDGE output), "AT" = fused ts=6,7 tail.
_SCHEDULE = [
    "D:32", "L:wq0", "W:0", "L:wka0", "L:wq1", "L:mask", "L:ident", "L:wq2",
    "L:wq3",
    "SP:0",
    "L:wkb0",
    "SP:2",
    "L:wka1",
    "SP:4",
    "L:wkb1",
    "SP:6",
    "L:wka2", "L:v8_0", "L:v8_1", "L:pp0",
    "SP:8",
    "L:wkb2", "L:v8_2",
    "A:0", "A:1",
    "SP:10",
    "L:wka3", "L:v8_3", "L:pp1",
    "A:2", "A:3",
    "SP:12",
    "L:wkb3", "L:v8_4", "L:v8_5", "L:pp2",
    "A:4",
    "SP:14",
    "L:pps", "L:ones", "L:v8_6", "L:v8_7",
    "AA:5",
    "AT:0",
]


def build_nc():
    """Build + schedule + compile the (single, SPMD-uniform) bass program."""
    import concourse.bass as bass
    import concourse.bacc as bacc
    import concourse.mybir as mybir
    import concourse.tile as tile

    fp32 = mybir.dt.float32
    fp16 = mybir.dt.float16
    fp8 = mybir.dt.float8e4
    AF = mybir.ActivationFunctionType
    DR = mybir.MatmulPerfMode.DoubleRow

    nc = bacc.Bacc("TRN2", target_bir_lowering=False, debug=False)

    # WQ[g] (moving, sq-weighted): q cols [512g, 512g+256).
    # WKA[g]/WKB[g] (stationary, sk-weighted): s-tiles 4g,4g+1 / 4g+2,4g+3.
    wq_d = [nc.dram_tensor(f"wq{g}", [128, DT, 256], fp8,
                           kind="ExternalInput").ap() for g in range(NG)]
    wka_d = [nc.dram_tensor(f"wka{g}", [128, DT, 256], fp8,
                            kind="ExternalInput").ap() for g in range(NG)]
    wkb_d = [nc.dram_tensor(f"wkb{g}", [128, DT, 256], fp8,
                            kind="ExternalInput").ap() for g in range(NG)]
    # v8[k]: [128 p, 2 pair, 1024 d] = V/2 rows s = 256k+128i+p, fp8
    v8_d = nc.dram_tensor("v8", [NK, 128, 2, D], fp8, kind="ExternalInput").ap()
    # pp[j]: [128, 1024] fp16 broadcast: P' for chunks 0..2 (chunk 3 via ppmm)
    pp_d = nc.dram_tensor("pp", [NG - 1, 128, D], fp16,
                          kind="ExternalInput").ap()
    # pps: [1, 2048] P' chunks 2,3 + ones16 [1, 128] for K=1 fold-in matmuls
    pps_d = nc.dram_tensor("pps", [1, 2 * D], fp16, kind="ExternalInput").ap()
    ones_d = nc.dram_tensor("ones16", [1, 128], fp16, kind="ExternalInput").ap()
    # mask8[p, wq, i, n]: -240 where masked else 0; applied as a 5th
    # DoubleRow matmul ident8^T @ mask8 (both i-slots sum -> -480 bias).
    mask_d = nc.dram_tensor("mask8", [128, 4, 2, 256], fp8,
                            kind="ExternalInput").ap()
    ident_d = nc.dram_tensor("ident8", [128, 2, 128], fp8,
                             kind="ExternalInput").ap()
    out_d = nc.dram_tensor("out", [NQ, D], fp16, kind="ExternalOutput").ap()

    with tile.TileContext(nc) as tc:
        with (
            tc.tile_pool(name="const", bufs=1) as constp,
            tc.tile_pool(name="wt", bufs=1) as wtp,
            tc.tile_pool(name="vv", bufs=1) as vvp,
            tc.tile_pool(name="at", bufs=1) as atp,
            tc.tile_pool(name="outb", bufs=6) as outp,
        ):
            # PSUM pools are phase-scoped: "ps" (scores) closes before "po"
            # (AV) opens, so each phase gets more concurrent banks.
            pools = {}
            # ---- constants ----
            mask_sb = constp.tile([128, 4, 2, 256], fp8, tag="mask8")
            ident_sb = constp.tile([128, 2, 128], fp8, tag="ident8")
            ppt = [constp.tile([128, D], fp16, tag=f"pp{j}", name=f"pp{j}")
                   for j in range(NG - 1)]
            pps_sb = constp.tile([1, 2 * D], fp16, tag="pps")
            ones_sb = constp.tile([1, 128], fp16, tag="ones16")
            warm_sb = constp.tile([128, 16], fp16, tag="warm")
            # zeroed operand for PE p-state pre-warm dummy matmuls
            dmw_sb = constp.tile([128, 2, 128], fp8, tag="dmw")

            # ---- persistent arrays ----
            wq = [wtp.tile([128, DT, 256], fp8, tag=f"wq{g}", name=f"wq{g}")
                  for g in range(NG)]
            wka = [wtp.tile([128, DT, 256], fp8, tag=f"wka{g}", name=f"wka{g}")
                   for g in range(NG)]
            wkb = [wtp.tile([128, DT, 256], fp8, tag=f"wkb{g}", name=f"wkb{g}")
                   for g in range(NG)]
            v8 = [vvp.tile([128, 2, D], fp8, tag=f"v8_{k}", name=f"v8_{k}")
                  for k in range(NK)]
            # attnT pair tiles: [128 s-part, 2 pair, 1024 q] fp8
            att = [atp.tile([128, 2, NQ], fp8, tag=f"att{k}", name=f"att{k}")
                   for k in range(NK)]

            def score_chain(ps, col, ss, gi, g):
                g0 = ss // 4
                r = ss % 4
                src = wka[g0] if r < 2 else wkb[g0]
                c0 = (r % 2) * 128
                for k in range(DT // 2):
                    nc.tensor.matmul(
                        ps[:, col:col + 256],
                        src[:, 2 * k:2 * k + 2, c0:c0 + 128],
                        wq[g][:, 2 * k:2 * k + 2, :],
                        start=(k == 0),
                        stop=(k == DT // 2 - 1) and gi != 0,
                        perf_mode=DR,
                    )
                if gi == 0:
                    # boundary chunk: additive -480 mask via one more matmul
                    nc.tensor.matmul(
                        ps[:, col:col + 256], ident_sb[:],
                        mask_sb[:, ss % 4, :, :],
                        start=False, stop=True, perf_mode=DR,
                    )

            def scores_pair(sa):
                """Two s-tiles (sa, sa+1), chains interleaved, with
                half-width psum tiles so Act frees them pipelined."""
                sb = sa + 1
                g0 = sa // 4
                nch = NG - g0
                nph = (nch + 1) // 2
                for ph in range(nph):
                    tiles = {}
                    for ss in (sa, sb):
                        tiles[ss] = pools["ps"].tile([128, 512], fp32,
                                                     tag="ps",
                                                     name=f"ps{ss}_{ph}")
                    for gi in (2 * ph, 2 * ph + 1):
                        if gi < nch:
                            for ss in (sa, sb):
                                score_chain(tiles[ss], (gi - 2 * ph) * 256,
                                            ss, gi, g0 + gi)
                    w = min(512, (nch - 2 * ph) * 256)
                    cb = (g0 + 2 * ph) * 256
                    for ss in (sa, sb):
                        nc.scalar.activation(
                            att[ss // 2][:, ss % 2, cb:cb + w],
                            tiles[ss][:, 0:w], AF.Tanh, scale=0.0625)

            def av_mm(po, ts, dh, k, nk, first_start=True):
                nc.tensor.matmul(
                    po[:],
                    att[k][:, :, ts * 128:(ts + 1) * 128],
                    v8[k][:, :, dh * 512:(dh + 1) * 512],
                    start=(k == 0) and first_start,
                    stop=(k == nk - 1),
                    perf_mode=DR,
                )

            def av(ts, act_fin=False):
                """output rows t=[128ts,128ts+128): accumulate over s prefix.

                act_fin: fold P' in with a K=1 matmul and finish on the Act
                engine (Copy) instead of a DVE add — balances DVE vs Act.
                """
                nk = 2 * (ts // 2 + 1)
                j = ts // 2
                ob = outp.tile([128, D], fp16, tag="ob", name=f"ob{ts}")
                for dh in range(2):
                    po = pools["po"].tile([128, 512], fp32, tag="po",
                                          name=f"po{ts}_{dh}")
                    for k in range(nk):
                        av_mm(po, ts, dh, k,
                              nk + (1 if act_fin else 0))
                    dsl = slice(dh * 512, (dh + 1) * 512)
                    dst = out_d[ts * 128:(ts + 1) * 128, dsl]
                    if act_fin:
                        nc.tensor.matmul(
                            po[:], ones_sb[:],
                            pps_sb[:, (j - 2) * D + dh * 512:
                                   (j - 2) * D + (dh + 1) * 512],
                            start=False, stop=True,
                        )
                        nc.scalar.activation(ob[:, dsl], po[:], AF.Copy)
                        nc.gpsimd.dma_start(dst, ob[:, dsl])
                    else:
                        nc.vector.tensor_add(ob[:, dsl], po[:], ppt[j][:, dsl])
                        nc.gpsimd.dma_start(dst, ob[:, dsl])

            def av_tail():
                """ts=6,7 with all four chains live so only the final k=7
                matmuls wait on the last v8 load. P' folds in via a K=1
                fp16 matmul; fins split across Act (copy) and DVE (copy)."""
                obs = {ts: outp.tile([128, D], fp16, tag="ob", name=f"ob{ts}")
                       for ts in (6, 7)}
                pos = {}
                for ts in (6, 7):
                    for dh in range(2):
                        pos[ts, dh] = pools["po"].tile([128, 512], fp32,
                                                       tag="po",
                                                       name=f"po{ts}_{dh}")
                for k in range(NK):
                    for ts in (6, 7):
                        for dh in range(2):
                            av_mm(pos[ts, dh], ts, dh, k,
                                  nk=NK + 1)  # stop deferred to ppmm
                for ts in (6, 7):
                    for dh in range(2):
                        # accumulate P'[3] broadcast: ones16^T @ pps (K=1)
                        nc.tensor.matmul(
                            pos[ts, dh][:], ones_sb[:],
                            pps_sb[:, D + dh * 512:D + (dh + 1) * 512],
                            start=False, stop=True,
                        )
                for ts in (6, 7):
                    for dh in range(2):
                        dst = obs[ts][:, dh * 512:(dh + 1) * 512]
                        if ts == 7:
                            nc.scalar.activation(dst, pos[ts, dh][:], AF.Copy)
                        else:
                            nc.vector.tensor_copy(dst, pos[ts, dh][:])
                for ts in (6, 7):
                    nc.sync.dma_start(out_d[ts * 128:(ts + 1) * 128, :],
                                      obs[ts][:])

            def load(sb, dram):
                nc.sync.dma_start(sb[:], dram)

            # ---- emission order (see _SCHEDULE) ----
            loadables = {
                "mask": (mask_sb, mask_d),
                "ident": (ident_sb, ident_d),
                **{f"wq{g}": (wq[g], wq_d[g]) for g in range(NG)},
                **{f"wka{g}": (wka[g], wka_d[g]) for g in range(NG)},
                **{f"wkb{g}": (wkb[g], wkb_d[g]) for g in range(NG)},
                **{f"v8_{k}": (v8[k], v8_d[k]) for k in range(NK)},
                **{f"pp{j}": (ppt[j], pp_d[j]) for j in range(NG - 1)},
                "pps": (pps_sb, pps_d),
                "ones": (ones_sb, ones_d),
            }
            import contextlib
            with contextlib.ExitStack() as stack:
                pools["ps"] = stack.enter_context(
                    tc.tile_pool(name="ps", bufs=4, space="PSUM"))
                pools["po"] = stack.enter_context(
                    tc.tile_pool(name="po", bufs=4, space="PSUM"))
                for step in _SCHEDULE:
                    kind, arg = step.split(":", 1)
                    if kind == "L":
                        load(*loadables[arg])
                    elif kind == "SP":
                        scores_pair(int(arg))
                    elif kind == "PH":
                        pass
                    elif kind == "A":
                        av(int(arg))
                    elif kind == "AA":
                        av(int(arg), act_fin=True)
                    elif kind == "AT":
                        av_tail()
                    elif kind == "W":
                        # preload the Tanh act table off the critical path
                        nc.scalar.activation(warm_sb[:], wq[0][:, 0, 0:16],
                                             AF.Tanh)
                    elif kind == "D":
                        # PE p-state pre-warm: dummy DoubleRow matmuls on
                        # zeroed tiles keep the clock ramping while the
                        # first weight DMAs are in flight.
                        n = int(arg)
                        nc.gpsimd.memset(dmw_sb[:], 0)
                        pd = pools["ps"].tile([128, 512], fp32, tag="ps",
                                              name="pswarm")
                        for i in range(n):
                            nc.tensor.matmul(
                                pd[:, 0:128], dmw_sb[:], dmw_sb[:],
                                start=(i == 0), stop=(i == n - 1),
                                perf_mode=DR,
                            )

    nc.compile()
    return nc


def host_inputs(x, bv_q, bv_k, bv_v):
    """Build per-core input maps (all host work is O(T*D) elementwise)."""
    import ml_dtypes

    f8 = ml_dtypes.float8_e4m3

    x = np.ascontiguousarray(np.asarray(x, dtype=np.float32))
    sq = np.sign(np.asarray(bv_q, dtype=np.float32))
    sk = np.sign(np.asarray(bv_k, dtype=np.float32))
    sv = np.sign(np.asarray(bv_v, dtype=np.float32))

    masks = {}
    for parity in (0, 1):
        wo = np.arange(512)[:, None]                     # boundary s offset
        ct = np.arange(256)[None, :]                     # q col offset in group
        if parity == 0:
            keep = wo <= ct                              # orig offsets equal
        else:
            so = np.where(wo < 256, wo + 256, wo - 256)  # swapped halves
            keep = so <= ct + 256
        mb = np.where(keep, np.float32(0), np.float32(-240.0))
        # [512, 256] -> [128 p, 4 wq, 256] -> duplicate into both i-slots
        m3 = mb.reshape(4, 128, 256).transpose(1, 0, 2)  # [128, 4, 256]
        masks[parity] = np.ascontiguousarray(
            np.broadcast_to(m3[:, :, None, :], (128, 4, 2, 256)))
    ident8 = np.zeros((128, 2, 128), np.float32)
    ident8[np.arange(128), :, np.arange(128)] = 1.0

    in_maps = []
    for core in range(NCORES):
        b, parity = core // 2, core % 2
        xb = x[b]
        if parity == 0:
            xkc = xb
        else:
            xkc = np.ascontiguousarray(
                xb.reshape(NG, 2, 256, D)[:, ::-1].reshape(T, D))
        # WQ = sq * S^T, WK = sk * S^T [D, T], blocked [dk, p, g, s-off]
        st = np.sign(xkc).T                              # [1024 d, 2048 s]
        wqr = (sq[:, None] * st).reshape(DT, 128, NG, 512)
        wkr = (sk[:, None] * st).reshape(DT, 128, NG, 512)
        v = xkc * sv                                     # [T, D] fp32
        # v8: [8 k, 128 p, 2 i, 1024] = V/2 at row 256k+128i+p
        v8 = np.ascontiguousarray(
            (0.5 * v).reshape(NK, 2, 128, D).transpose(0, 2, 1, 3)).astype(f8)
        # P'[j] = 0.5 * sum_{s < 512(j+1)} V[s]; broadcast to 128 partitions
        cs = np.cumsum(v, axis=0)
        ppj = 0.5 * cs[512 * np.arange(1, NG + 1) - 1]   # [4, 1024]
        pp = np.ascontiguousarray(np.broadcast_to(
            ppj[:NG - 1, None].astype(np.float16), (NG - 1, 128, D)))
        m = {"v8": v8, "pp": pp, "mask8": masks[parity].astype(f8),
             "ident8": ident8.astype(f8),
             "pps": np.ascontiguousarray(
                 ppj[NG - 2:NG].reshape(1, 2 * D)).astype(np.float16),
             "ones16": np.ones((1, 128), np.float16)}
        for g in range(NG):
            m[f"wq{g}"] = np.ascontiguousarray(
                wqr[:, :, g, 0:256].transpose(1, 0, 2)).astype(f8)
            m[f"wka{g}"] = np.ascontiguousarray(
                wkr[:, :, g, 0:256].transpose(1, 0, 2)).astype(f8)
            m[f"wkb{g}"] = np.ascontiguousarray(
                wkr[:, :, g, 256:512].transpose(1, 0, 2)).astype(f8)
        in_maps.append(m)
    return in_maps


def assemble_output(results):
    out = np.zeros((B, T, D), np.float32)
    for core in range(NCORES):
        b, parity = core // 2, core % 2
        o = np.asarray(results[core]["out"]).astype(np.float32).reshape(NG, 256, D)
        for j in range(NG):
            r0 = 512 * j + 256 * parity
            out[b, r0:r0 + 256] = o[j]
    return out


def kernel(x, bv_q, bv_k, bv_v):
    from concourse.bass_utils import run_bass_kernel_spmd

    if "nc" not in _CACHE:
        _CACHE["nc"] = build_nc()
    nc = _CACHE["nc"]

    in_maps = host_inputs(x, bv_q, bv_k, bv_v)
    res = run_bass_kernel_spmd(nc, in_maps, list(range(NCORES)))
    _CACHE["last_result"] = res
    return assemble_output(res.results)
